# revision 1
# baseline (speedup 1.0000x reference)
"""Trainium2 Bass kernel for nn_KVEmbedding (embedding_lookup).

reference: out[b, l, :] = table[indices[b, l], :]
  indices: (4096, 200) int in [0, 1M); table: (1M, 64) f32
  out: (4096, 200, 64) f32

Strategy (8 NeuronCores): data-parallel over the batch dim — each core gets
512 of the 4096 index rows (102,400 lookups) and a full table replica in its
HBM. No collectives. Per core the output rows r = p*800 + g map to SBUF
partition p, free slot g; gathers fill [128, CHUNK*64] SBUF tiles which are
written back with 25.6 KB/partition contiguous descriptors.

MODE selects the gather formulation (HW-validated via probes):
  flat_interleaved: offset AP [1, N] per chunk; flat element i -> dst
      partition i%128, slot i//128 (host pre-permutes indices to match).
  flat_partmajor: offset AP [1, N] per chunk; element p*CHUNK+j -> dst
      (p, j) (sim/C-order semantics).
  rows128: CHUNK indirect DMAs of [128, 1] per chunk (known-good on HW,
      higher Q7 descriptor-gen overhead).
"""

import numpy as np

N_CORES = 8
B, L = 4096, 200
V, D = 1_000_000, 64
P = 128
ROWS_PER_CORE = B * L // N_CORES  # 102400
G = ROWS_PER_CORE // P  # 800 lookups per partition
CHUNK = 100  # slots per partition per chunk
NCHUNK = G // CHUNK  # 8
NPC = P * CHUNK  # 12800 lookups per chunk

MODE = "rows128"  # updated after HW probes

_NC_CACHE: dict = {}


def build_nc(mode=None, bufs=3):
    mode = mode or MODE
    from concourse import bass, mybir
    import concourse.bacc as bacc
    import concourse.tile as tile

    nc = bacc.Bacc(
        "TRN2", target_bir_lowering=False, debug=False, num_devices=N_CORES
    )
    table_t = nc.dram_tensor("table", [V, D], mybir.dt.float32, kind="ExternalInput")
    if mode.startswith("flat"):
        idx_t = nc.dram_tensor(
            "idx", [NCHUNK, NPC], mybir.dt.int32, kind="ExternalInput"
        )
    else:
        idx_t = nc.dram_tensor("idx", [P, G], mybir.dt.int32, kind="ExternalInput")
    out_t = nc.dram_tensor(
        "out", [ROWS_PER_CORE, D], mybir.dt.float32, kind="ExternalOutput"
    )

    with tile.TileContext(nc) as tc:
        with (
            tc.tile_pool(name="idxp", bufs=1) as ipool,
            tc.tile_pool(name="gath", bufs=bufs) as gpool,
        ):
            if mode.startswith("flat"):
                idx_sb = ipool.tile([NCHUNK, NPC], mybir.dt.int32)
            else:
                idx_sb = ipool.tile([P, G], mybir.dt.int32)
            nc.sync.dma_start(out=idx_sb[:], in_=idx_t.ap())

            out_view = out_t.ap().rearrange("(p g) d -> p g d", p=P)
            for c in range(NCHUNK):
                gt = gpool.tile([P, CHUNK * D], mybir.dt.float32, tag="gt")
                if mode.startswith("flat"):
                    nc.gpsimd.indirect_dma_start(
                        out=gt[:],
                        out_offset=None,
                        in_=table_t.ap(),
                        in_offset=bass.IndirectOffsetOnAxis(
                            ap=idx_sb[c : c + 1, :], axis=0
                        ),
                    )
                else:  # rows128
                    for g in range(CHUNK):
                        nc.gpsimd.indirect_dma_start(
                            out=gt[:, g * D : (g + 1) * D],
                            out_offset=None,
                            in_=table_t.ap(),
                            in_offset=bass.IndirectOffsetOnAxis(
                                ap=idx_sb[:, c * CHUNK + g : c * CHUNK + g + 1],
                                axis=0,
                            ),
                        )
                nc.sync.dma_start(
                    out=out_view[:, c * CHUNK : (c + 1) * CHUNK, :], in_=gt[:]
                )

    nc.compile()
    return nc


def _get_nc():
    if "nc" not in _NC_CACHE:
        _NC_CACHE["nc"] = build_nc()
    return _NC_CACHE["nc"]


def make_in_maps(indices: np.ndarray, table: np.ndarray, mode=None) -> list[dict]:
    mode = mode or MODE
    idx = np.ascontiguousarray(indices.astype(np.int32, copy=False)).reshape(
        N_CORES, P, NCHUNK, CHUNK
    )  # [core, p, c, j] = flat[core, p*800 + c*100 + j]
    table = np.ascontiguousarray(table.astype(np.float32, copy=False))
    maps = []
    for i in range(N_CORES):
        if mode == "flat_interleaved":
            # element i=j*128+p of chunk c -> dst(p, j): idx_dram[c, j*128+p]
            # idx[i] is [p, c, j]; -> [c, j, p] so element (c, j*128+p) = idx[p, c, j]
            a = idx[i].transpose(1, 2, 0).reshape(NCHUNK, NPC)
            maps.append({"table": table, "idx": np.ascontiguousarray(a)})
        elif mode == "flat_partmajor":
            # element p*CHUNK+j of chunk c -> dst(p, j): idx_dram[c, p*CHUNK+j]
            a = idx[i].transpose(1, 0, 2).reshape(NCHUNK, NPC)  # [c, p, j]
            maps.append({"table": table, "idx": np.ascontiguousarray(a)})
        else:  # rows128
            a = idx[i].reshape(P, G)
            maps.append({"table": table, "idx": np.ascontiguousarray(a)})
    return maps


def assemble_out(results: list[dict]) -> np.ndarray:
    outs = [results[i]["out"].reshape(B // N_CORES, L, D) for i in range(N_CORES)]
    return np.concatenate(outs, axis=0)


def run_on_hw(indices: np.ndarray, table: np.ndarray, **spmd_kwargs):
    from concourse.bass_utils import run_bass_kernel_spmd

    nc = _get_nc()
    in_maps = make_in_maps(indices, table)
    res = run_bass_kernel_spmd(
        nc, in_maps, core_ids=list(range(N_CORES)), **spmd_kwargs
    )
    return assemble_out(res.results), res


def kernel(indices: np.ndarray, table: np.ndarray, dummy=None, **_unused) -> np.ndarray:
    out, _ = run_on_hw(np.asarray(indices), np.asarray(table))
    return out



# revision 6
# speedup vs baseline: 1.0362x; 1.0362x over previous
"""Trainium2 Bass kernel for nn_KVEmbedding (embedding_lookup).

reference: out[b, l, :] = table[indices[b, l], :]
  indices: (4096, 200) int in [0, 1M); table: (1M, 64) f32
  out: (4096, 200, 64) f32

Strategy (8 NeuronCores): data-parallel over the batch dim — each core gets
512 of the 4096 index rows (102,400 lookups) and a full table replica in its
HBM. No collectives. Per core the output rows r = p*800 + g map to SBUF
partition p, free slot g; indirect DMAs gather 128 rows each ([128, 1] offset
AP = one offset per partition), staged through small SBUF tiles and written
back with contiguous descriptors.

HW findings driving this shape (validated by identity-table probes):
  - indirect_dma_start with a MULTI-offset AP ([128, k>1] or [1, N]) does NOT
    work on this hardware/ucode build: only the first offset per partition is
    honored, with the dst extent filled from contiguous table rows (a probe
    with consecutive row ids per partition masks this, so beware false
    positives). [1, N] offset APs with N*16B beyond the dynamic-DMA scratch
    crash the runtime outright.
  - The [128, 1] offset form (one row per partition per instruction) is
    correct and is the only usable gather shape, so the kernel issues 800
    such instructions per core. Per-instruction SWDGE descriptor generation
    (~1.04 us fixed, serial on the Pool engine) is then the wall: ~830 us.
  - Small staging chunks (8 gathers per writeback tile) with a deep pool and
    a split index load shave the remaining pipeline stalls: ~839 us vs
    869 us for 100-row chunks.
"""

import numpy as np

N_CORES = 8
B, L = 4096, 200
V, D = 1_000_000, 64
P = 128
ROWS_PER_CORE = B * L // N_CORES  # 102400
G = ROWS_PER_CORE // P  # 800 lookups per partition
CHUNK = 8  # rows per partition staged per writeback tile
NCHUNK = G // CHUNK  # 100
BUFS = 12

_NC_CACHE: dict = {}


def build_nc(chunk=CHUNK, bufs=BUFS):
    from concourse import bass, mybir
    import concourse.bacc as bacc
    import concourse.tile as tile

    nchunk = G // chunk
    nc = bacc.Bacc(
        "TRN2", target_bir_lowering=False, debug=False, num_devices=N_CORES
    )
    table_t = nc.dram_tensor("table", [V, D], mybir.dt.float32, kind="ExternalInput")
    idx_t = nc.dram_tensor("idx", [P, G], mybir.dt.int32, kind="ExternalInput")
    out_t = nc.dram_tensor(
        "out", [ROWS_PER_CORE, D], mybir.dt.float32, kind="ExternalOutput"
    )

    with tile.TileContext(nc) as tc:
        with (
            tc.tile_pool(name="idxp", bufs=1) as ipool,
            tc.tile_pool(name="gath", bufs=bufs) as gpool,
        ):
            idx_sb = ipool.tile([P, G], mybir.dt.int32)
            iv = idx_t.ap()
            # split load: chunk 0's offsets land first so gathers start sooner
            nc.sync.dma_start(out=idx_sb[:, :chunk], in_=iv[:, :chunk])
            nc.sync.dma_start(out=idx_sb[:, chunk:], in_=iv[:, chunk:])

            out_view = out_t.ap().rearrange("(p g) d -> p g d", p=P)
            for c in range(nchunk):
                gt = gpool.tile([P, chunk * D], mybir.dt.float32, tag="gt")
                for g in range(chunk):
                    nc.gpsimd.indirect_dma_start(
                        out=gt[:, g * D : (g + 1) * D],
                        out_offset=None,
                        in_=table_t.ap(),
                        in_offset=bass.IndirectOffsetOnAxis(
                            ap=idx_sb[:, c * chunk + g : c * chunk + g + 1],
                            axis=0,
                        ),
                    )
                nc.sync.dma_start(
                    out=out_view[:, c * chunk : (c + 1) * chunk, :], in_=gt[:]
                )

    nc.compile()
    return nc


def _get_nc():
    if "nc" not in _NC_CACHE:
        _NC_CACHE["nc"] = build_nc()
    return _NC_CACHE["nc"]


def make_in_maps(indices: np.ndarray, table: np.ndarray) -> list[dict]:
    idx = np.ascontiguousarray(indices.astype(np.int32, copy=False)).reshape(
        N_CORES, P, G
    )  # [core, p, g] = flat[core, p*G + g]
    table = np.ascontiguousarray(np.asarray(table, dtype=np.float32))
    return [
        {"table": table, "idx": np.ascontiguousarray(idx[i])}
        for i in range(N_CORES)
    ]


def assemble_out(results: list[dict]) -> np.ndarray:
    outs = [results[i]["out"].reshape(B // N_CORES, L, D) for i in range(N_CORES)]
    return np.concatenate(outs, axis=0)


def run_on_hw(indices: np.ndarray, table: np.ndarray, **spmd_kwargs):
    from concourse.bass_utils import run_bass_kernel_spmd

    nc = _get_nc()
    in_maps = make_in_maps(indices, table)
    res = run_bass_kernel_spmd(
        nc, in_maps, core_ids=list(range(N_CORES)), **spmd_kwargs
    )
    return assemble_out(res.results), res


def kernel(indices: np.ndarray, table: np.ndarray, dummy=None, **_unused) -> np.ndarray:
    out, _ = run_on_hw(np.asarray(indices), np.asarray(table))
    return out


# revision 8
# speedup vs baseline: 1.0369x; 1.0007x over previous
"""Trainium2 Bass kernel for nn_KVEmbedding (embedding_lookup).

reference: out[b, l, :] = table[indices[b, l], :]
  indices: (4096, 200) int in [0, 1M); table: (1M, 64) f32
  out: (4096, 200, 64) f32

Strategy (8 NeuronCores): data-parallel over the batch dim — each core gets
512 of the 4096 index rows (102,400 lookups) and a full table replica in its
HBM. No collectives. Per core the output rows r = p*800 + g map to SBUF
partition p, free slot g; indirect DMAs gather 128 rows each ([128, 1] offset
AP = one offset per partition), staged through small SBUF tiles and written
back with contiguous descriptors.

HW findings driving this shape (validated by identity-table probes):
  - indirect_dma_start with a MULTI-offset AP ([128, k>1] or [1, N]) does NOT
    work on this hardware/ucode build: only the first offset per partition is
    honored, with the dst extent filled from contiguous table rows (a probe
    with consecutive row ids per partition masks this, so beware false
    positives). [1, N] offset APs with N*16B beyond the dynamic-DMA scratch
    crash the runtime outright.
  - The [128, 1] offset form (one row per partition per instruction) is
    correct and is the only usable gather shape, so the kernel issues 800
    such instructions per core. Per-instruction SWDGE descriptor generation
    (~1.04 us fixed, serial on the Pool engine) is then the wall: ~830 us.
  - Small staging chunks (4 gathers per writeback tile, 1-row drain tail)
    with a deep pool and a split index load shave the remaining pipeline
    stalls: ~838 us vs 869 us for 100-row chunks. Multi-offset forms were
    also re-probed as whole-tile offset APs and 3D dst APs — all broken.
"""

import numpy as np

N_CORES = 8
B, L = 4096, 200
V, D = 1_000_000, 64
P = 128
ROWS_PER_CORE = B * L // N_CORES  # 102400
G = ROWS_PER_CORE // P  # 800 lookups per partition
# 4 rows/partition per staging tile keeps the Pool engine streaming with
# minimal writeback stalls; the trailing 1-row chunks shorten the drain tail.
SCHEDULE = [4] * 199 + [1] * 4
BUFS = 24

_NC_CACHE: dict = {}


def build_nc(schedule=None, bufs=BUFS):
    from concourse import bass, mybir
    import concourse.bacc as bacc
    import concourse.tile as tile

    schedule = schedule or SCHEDULE
    assert sum(schedule) == G
    nc = bacc.Bacc(
        "TRN2", target_bir_lowering=False, debug=False, num_devices=N_CORES
    )
    table_t = nc.dram_tensor("table", [V, D], mybir.dt.float32, kind="ExternalInput")
    idx_t = nc.dram_tensor("idx", [P, G], mybir.dt.int32, kind="ExternalInput")
    out_t = nc.dram_tensor(
        "out", [ROWS_PER_CORE, D], mybir.dt.float32, kind="ExternalOutput"
    )

    with tile.TileContext(nc) as tc:
        with (
            tc.tile_pool(name="idxp", bufs=1) as ipool,
            tc.tile_pool(name="gath", bufs=bufs) as gpool,
        ):
            idx_sb = ipool.tile([P, G], mybir.dt.int32)
            iv = idx_t.ap()
            # split load: chunk 0's offsets land first so gathers start sooner
            c0 = schedule[0]
            nc.sync.dma_start(out=idx_sb[:, :c0], in_=iv[:, :c0])
            nc.sync.dma_start(out=idx_sb[:, c0:], in_=iv[:, c0:])

            out_view = out_t.ap().rearrange("(p g) d -> p g d", p=P)
            pos = 0
            for chunk in schedule:
                gt = gpool.tile([P, chunk * D], mybir.dt.float32, tag="gt")
                for g in range(chunk):
                    nc.gpsimd.indirect_dma_start(
                        out=gt[:, g * D : (g + 1) * D],
                        out_offset=None,
                        in_=table_t.ap(),
                        in_offset=bass.IndirectOffsetOnAxis(
                            ap=idx_sb[:, pos + g : pos + g + 1],
                            axis=0,
                        ),
                    )
                nc.sync.dma_start(
                    out=out_view[:, pos : pos + chunk, :], in_=gt[:]
                )
                pos += chunk

    nc.compile()
    return nc


def _get_nc():
    if "nc" not in _NC_CACHE:
        _NC_CACHE["nc"] = build_nc()
    return _NC_CACHE["nc"]


def make_in_maps(indices: np.ndarray, table: np.ndarray) -> list[dict]:
    idx = np.ascontiguousarray(indices.astype(np.int32, copy=False)).reshape(
        N_CORES, P, G
    )  # [core, p, g] = flat[core, p*G + g]
    table = np.ascontiguousarray(np.asarray(table, dtype=np.float32))
    return [
        {"table": table, "idx": np.ascontiguousarray(idx[i])}
        for i in range(N_CORES)
    ]


def assemble_out(results: list[dict]) -> np.ndarray:
    outs = [results[i]["out"].reshape(B // N_CORES, L, D) for i in range(N_CORES)]
    return np.concatenate(outs, axis=0)


def run_on_hw(indices: np.ndarray, table: np.ndarray, **spmd_kwargs):
    from concourse.bass_utils import run_bass_kernel_spmd

    nc = _get_nc()
    in_maps = make_in_maps(indices, table)
    res = run_bass_kernel_spmd(
        nc, in_maps, core_ids=list(range(N_CORES)), **spmd_kwargs
    )
    return assemble_out(res.results), res


def kernel(indices: np.ndarray, table: np.ndarray, dummy=None, **_unused) -> np.ndarray:
    out, _ = run_on_hw(np.asarray(indices), np.asarray(table))
    return out


# revision 9
# speedup vs baseline: 1.0371x; 1.0002x over previous
"""Trainium2 Bass kernel for nn_KVEmbedding (embedding_lookup).

reference: out[b, l, :] = table[indices[b, l], :]
  indices: (4096, 200) int in [0, 1M); table: (1M, 64) f32
  out: (4096, 200, 64) f32

Strategy (8 NeuronCores): data-parallel over the batch dim — each core gets
512 of the 4096 index rows (102,400 lookups) and a full table replica in its
HBM. No collectives. Per core the output rows r = p*800 + g map to SBUF
partition p, free slot g; indirect DMAs gather 128 rows each ([128, 1] offset
AP = one offset per partition), staged through small SBUF tiles and written
back with contiguous descriptors.

HW findings driving this shape (validated by identity-table probes):
  - indirect_dma_start with a MULTI-offset AP ([128, k>1] or [1, N]) does NOT
    work on this hardware/ucode build: only the first offset per partition is
    honored, with the dst extent filled from contiguous table rows (a probe
    with consecutive row ids per partition masks this, so beware false
    positives). [1, N] offset APs with N*16B beyond the dynamic-DMA scratch
    crash the runtime outright.
  - The [128, 1] offset form (one row per partition per instruction) is
    correct and is the only usable gather shape, so the kernel issues 800
    such instructions per core. Per-instruction SWDGE descriptor generation
    (~1.04 us fixed, serial on the Pool engine) is then the wall: ~830 us.
  - Small staging chunks (4 gathers per writeback tile, 1-row drain tail)
    with a deep pool and a split index load shave the remaining pipeline
    stalls: ~838 us vs 869 us for 100-row chunks. Multi-offset forms were
    also re-probed as whole-tile offset APs and 3D dst APs — all broken.
"""

import numpy as np

N_CORES = 8
B, L = 4096, 200
V, D = 1_000_000, 64
P = 128
ROWS_PER_CORE = B * L // N_CORES  # 102400
G = ROWS_PER_CORE // P  # 800 lookups per partition
# 4 rows/partition per staging tile keeps the Pool engine streaming with
# minimal writeback stalls; the trailing 1-row chunks shorten the drain tail.
SCHEDULE = [4] * 198 + [1] * 8
BUFS = 24

_NC_CACHE: dict = {}


def build_nc(schedule=None, bufs=BUFS):
    from concourse import bass, mybir
    import concourse.bacc as bacc
    import concourse.tile as tile

    schedule = schedule or SCHEDULE
    assert sum(schedule) == G
    nc = bacc.Bacc(
        "TRN2", target_bir_lowering=False, debug=False, num_devices=N_CORES
    )
    table_t = nc.dram_tensor("table", [V, D], mybir.dt.float32, kind="ExternalInput")
    idx_t = nc.dram_tensor("idx", [P, G], mybir.dt.int32, kind="ExternalInput")
    out_t = nc.dram_tensor(
        "out", [ROWS_PER_CORE, D], mybir.dt.float32, kind="ExternalOutput"
    )

    with tile.TileContext(nc) as tc:
        with (
            tc.tile_pool(name="idxp", bufs=1) as ipool,
            tc.tile_pool(name="gath", bufs=bufs) as gpool,
        ):
            idx_sb = ipool.tile([P, G], mybir.dt.int32)
            iv = idx_t.ap()
            # split load: chunk 0's offsets land first so gathers start sooner
            c0 = schedule[0]
            nc.sync.dma_start(out=idx_sb[:, :c0], in_=iv[:, :c0])
            nc.sync.dma_start(out=idx_sb[:, c0:], in_=iv[:, c0:])

            out_view = out_t.ap().rearrange("(p g) d -> p g d", p=P)
            pos = 0
            for chunk in schedule:
                gt = gpool.tile([P, chunk * D], mybir.dt.float32, tag="gt")
                for g in range(chunk):
                    nc.gpsimd.indirect_dma_start(
                        out=gt[:, g * D : (g + 1) * D],
                        out_offset=None,
                        in_=table_t.ap(),
                        in_offset=bass.IndirectOffsetOnAxis(
                            ap=idx_sb[:, pos + g : pos + g + 1],
                            axis=0,
                        ),
                    )
                nc.sync.dma_start(
                    out=out_view[:, pos : pos + chunk, :], in_=gt[:]
                )
                pos += chunk

    nc.compile()
    return nc


def _get_nc():
    if "nc" not in _NC_CACHE:
        _NC_CACHE["nc"] = build_nc()
    return _NC_CACHE["nc"]


def make_in_maps(indices: np.ndarray, table: np.ndarray) -> list[dict]:
    idx = np.ascontiguousarray(indices.astype(np.int32, copy=False)).reshape(
        N_CORES, P, G
    )  # [core, p, g] = flat[core, p*G + g]
    table = np.ascontiguousarray(np.asarray(table, dtype=np.float32))
    return [
        {"table": table, "idx": np.ascontiguousarray(idx[i])}
        for i in range(N_CORES)
    ]


def assemble_out(results: list[dict]) -> np.ndarray:
    outs = [results[i]["out"].reshape(B // N_CORES, L, D) for i in range(N_CORES)]
    return np.concatenate(outs, axis=0)


def run_on_hw(indices: np.ndarray, table: np.ndarray, **spmd_kwargs):
    from concourse.bass_utils import run_bass_kernel_spmd

    nc = _get_nc()
    in_maps = make_in_maps(indices, table)
    res = run_bass_kernel_spmd(
        nc, in_maps, core_ids=list(range(N_CORES)), **spmd_kwargs
    )
    return assemble_out(res.results), res


def kernel(indices: np.ndarray, table: np.ndarray, dummy=None, **_unused) -> np.ndarray:
    out, _ = run_on_hw(np.asarray(indices), np.asarray(table))
    return out
